# revision 1
# baseline (speedup 1.0000x reference)
"""MultiHeadGeneralizedPooling Trainium2 kernel.

Data-parallel over batch: 32 batches -> 8 cores x 4 batches.
Per core, everything is computed in "feature-major" layout (feature dim on
SBUF partitions, sequence on the free axis):

  Hi^T (d, s)  = P_cat^T @ X^T          TensorE bf16, PSUM; +P_b on copy-out
  A1^T (dh, s) = relu(W1aug^T @ Hi^T)   K=97 (97th row = ones -> W1 bias)
  A2^T (d, s)  = W2^T @ A1^T            accumulated over 3 k-tiles
  E            = exp(A2^T)              ScalarE; accum_out -> Z per partition
                                        (W2 bias dropped: softmax-invariant)
  u[d]         = sum_s E * Hi           DVE scalar_tensor_tensor accum,
                                        chunked per s-quarter (short tail)
  v            = u / Z                  one tiny (96, 32) multiply at the end

Host side pre-transposes/casts X to X^T bf16 and packs the (tiny) weights
into lhsT layouts, so the device does no transposes of the big tensor.
"""

import numpy as np
from contextlib import ExitStack

B, S, T = 32, 2048, 768
NH, DH, DHID = 8, 96, 384
NCORES = 8
BPC = B // NCORES  # batches per core
KT = T // 128      # 6 contraction tiles
DT = (NH * DH) // 128  # 6 d-tiles of the packed head dim
SC = 4             # s-chunks per batch
SCW = S // SC      # 512
KC = DHID // 128   # 3
import os
N_WARM = int(os.environ.get("K_NWARM", "56"))  # PE clock-gate warmup matmuls
FAST_WARM = os.environ.get("K_FASTWARM", "1") == "1"
LEAD = int(os.environ.get("K_LEAD", "8"))

_NC_CACHE = {}


def _segs():
    """Per projection d-tile: (psum_row, head, head_row, nrows) segments
    mapping packed d rows (128*dt + p) onto per-head (h, q<96) layout."""
    segs = []
    for dt in range(DT):
        cur, d0, d1 = [], 128 * dt, 128 * (dt + 1)
        d = d0
        while d < d1:
            h, q = d // DH, d % DH
            n = min(d1 - d, DH - q)
            cur.append((d - d0, h, q, n))
            d += n
        segs.append(cur)
    return segs


def _build_nc():
    import concourse.bacc as bacc
    import concourse.tile as tile
    from concourse import mybir

    f32 = mybir.dt.float32
    bf16 = mybir.dt.bfloat16
    AF = mybir.ActivationFunctionType
    OP = mybir.AluOpType
    AX = mybir.AxisListType

    nc = bacc.Bacc()
    xt = nc.declare_dram_parameter("xt", [BPC, KT, 128, S], bf16, isOutput=False)
    p_l = nc.declare_dram_parameter("p_l", [128, KT, NH * DH], bf16, isOutput=False)
    w1 = nc.declare_dram_parameter("w1", [DH, NH, DHID], bf16, isOutput=False)
    b1l = nc.declare_dram_parameter("b1l", [128, NH, KC], f32, isOutput=False)
    w2 = nc.declare_dram_parameter("w2", [128, NH, KC, DH], bf16, isOutput=False)
    pb = nc.declare_dram_parameter("pb", [128, DT], f32, isOutput=False)
    ident = nc.declare_dram_parameter("ident", [DH, DH], f32, isOutput=False)
    out = nc.declare_dram_parameter("out", [BPC * NH, DH], f32, isOutput=True)

    segs = _segs()

    with tile.TileContext(nc) as tc:
        with ExitStack() as ctx:
            singles = ctx.enter_context(tc.tile_pool(name="singles", bufs=1))
            xt_pool = ctx.enter_context(tc.tile_pool(name="xtp", bufs=2))
            flat_pool = ctx.enter_context(tc.tile_pool(name="flat", bufs=7))
            a1sb_pool = ctx.enter_context(tc.tile_pool(name="a1sb", bufs=4))
            e_pool = ctx.enter_context(tc.tile_pool(name="ep", bufs=3))
            stt_pool = ctx.enter_context(tc.tile_pool(name="sttp", bufs=3))
            small_pool = ctx.enter_context(tc.tile_pool(name="small", bufs=4))
            pp_pool = ctx.enter_context(tc.tile_pool(name="pp", bufs=2, space="PSUM"))
            a1p_pool = ctx.enter_context(tc.tile_pool(name="a1p", bufs=4, space="PSUM"))
            a2p_pool = ctx.enter_context(tc.tile_pool(name="a2p", bufs=2, space="PSUM"))

            # PE warmup: dummy matmuls while DMAs stream in. With FAST_WARM
            # the warm tiles are never written (values irrelevant, psum is
            # never read), so the PE starts the moment its queue opens.
            warm_sb = singles.tile([128, 128], bf16)
            warm_sb2 = singles.tile([128, 128], bf16)
            if not FAST_WARM:
                nc.vector.memset(warm_sb, 0.0)
                nc.vector.memset(warm_sb2, 0.0)
            for i in range(N_WARM):
                wp = pp_pool.tile([128, SCW], f32, tag="pp")
                nc.tensor.matmul(
                    wp[:, 0:128], warm_sb, warm_sb2, start=True, stop=True
                )
            if FAST_WARM:
                # Written only after all warm reads (WAR, not RAW): satisfies
                # the tile allocator without gating the PE on another engine.
                nc.gpsimd.memset(warm_sb, 0.0)
                nc.gpsimd.memset(warm_sb2, 0.0)

            # Projection inputs first (needed immediately). Few, large DMAs:
            # the Sync engine's ~0.75us per-descriptor issue cost dominates
            # early latency, not wire time (packets fan out over 16 engines).
            p_sb = singles.tile([128, KT, NH * DH], bf16)
            xt_t0 = xt_pool.tile([128, KT, S], bf16, tag="xt0")
            # interleave per-kt p_l and xt first-half issues: the first
            # projection matmul needs only the kt0 pair (~1.4us of wire), so
            # real work can begin at DMA pace instead of after the full load
            for kt in range(KT):
                nc.sync.dma_start(out=p_sb[:, kt, :], in_=p_l[:, kt, :])
                nc.sync.dma_start(out=xt_t0[:, kt, 0:1024], in_=xt[0, kt, :, 0:1024])
            # tiny constants next: the first proj copies need pb and the
            # first MLP units need w1/b1l well before the xt second halves
            pb_sb = singles.tile([128, DT], f32)
            nc.sync.dma_start(out=pb_sb, in_=pb[:])
            b1l_sb = singles.tile([128, NH, KC], f32)
            nc.sync.dma_start(out=b1l_sb, in_=b1l[:])
            w1_sb = singles.tile([DH, NH, DHID], bf16)
            nc.sync.dma_start(out=w1_sb, in_=w1[:])
            for kt in range(KT):
                nc.sync.dma_start(out=xt_t0[:, kt, 1024:S], in_=xt[0, kt, :, 1024:S])
            w2_sb = singles.tile([128, NH, KC, DH], bf16)
            nc.sync.dma_start(out=w2_sb, in_=w2[:])
            id_sb = singles.tile([DH, DH], f32)
            nc.sync.dma_start(out=id_sb, in_=ident[:])
            v_sb = singles.tile([DH, BPC * NH, SC], f32)
            zr_sb = singles.tile([DH, BPC * NH], f32)

            # Hi^T in per-head layout; row 96 is a constant ones row that
            # realizes the W1 bias as a 97th contraction row. Two manual
            # buffers so batch b+1's projection overlaps batch b's MLP.
            # Hi^T per-head buffers; the W1 bias is folded into the relu
            # copy-out (ACT bias / DVE two-op tensor_scalar), so no ones row.
            hh = []
            for i in range(2):
                t = singles.tile([DH, NH, S], bf16, tag=f"hh{i}", name=f"hh{i}")
                hh.append(t)

            def proj_quarter(b, xt_t, dt, sc, flat_t):
                ps = pp_pool.tile([128, SCW], f32, tag="pp")
                for kt in range(KT):
                    nc.tensor.matmul(
                        ps,
                        p_sb[:, kt, 128 * dt:128 * (dt + 1)],
                        xt_t[:, kt, SCW * sc:SCW * (sc + 1)],
                        start=(kt == 0),
                        stop=(kt == KT - 1),
                    )
                if (b == 0 and dt < 2) or (dt == 5 and sc % 2 == 0):
                    nc.scalar.activation(
                        out=flat_t[:, SCW * sc:SCW * (sc + 1)],
                        in_=ps,
                        func=AF.Identity,
                        bias=pb_sb[:, dt:dt + 1],
                    )
                else:
                    nc.vector.tensor_scalar_add(
                        out=flat_t[:, SCW * sc:SCW * (sc + 1)],
                        in0=ps,
                        scalar1=pb_sb[:, dt:dt + 1],
                    )

            def emit_remap_dtile(b, dt, scp, flat_t):
                hcur = hh[b % 2]
                s0, s1 = 2 * SCW * scp, 2 * SCW * (scp + 1)
                for (r0, h, q0, n) in segs[dt]:
                    nc.sync.dma_start(
                        out=hcur[q0:q0 + n, h, s0:s1],
                        in_=flat_t[r0:r0 + n, s0:s1],
                    )

            def mlp_quarter(b, h, sc, e_t, zp, flush=False):
                hcur = hh[b % 2]
                col = b * NH + h
                a1sb = a1sb_pool.tile([128, KC, SCW], bf16)
                for c in range(KC):
                    a1p = a1p_pool.tile([128, SCW], f32, tag="a1p")
                    nc.tensor.matmul(
                        a1p,
                        w1_sb[:, h, 128 * c:128 * (c + 1)],
                        hcur[:, h, SCW * sc:SCW * (sc + 1)],
                        start=True,
                        stop=True,
                    )
                    if flush:
                        on_act = c == 0 or (c == 1 and sc == 1)
                    else:
                        on_act = c == 0 or (c == 1 and sc != 3)
                    if on_act:
                        nc.scalar.activation(
                            out=a1sb[:, c, :], in_=a1p, func=AF.Relu,
                            bias=b1l_sb[:, h, c:c + 1],
                        )
                    else:
                        nc.vector.tensor_scalar(
                            out=a1sb[:, c, :], in0=a1p,
                            scalar1=b1l_sb[:, h, c:c + 1], scalar2=0.0,
                            op0=OP.add, op1=OP.max,
                        )
                a2p = a2p_pool.tile([DH, SCW], f32, tag="a2p")
                for kc in range(KC):
                    nc.tensor.matmul(
                        a2p,
                        w2_sb[:, h, kc, :],
                        a1sb[:, kc, :],
                        start=(kc == 0),
                        stop=(kc == KC - 1),
                    )
                hc = sc % 2  # chunk within the half-S e tile
                nc.scalar.activation(
                    out=e_t[:, SCW * hc:SCW * (hc + 1)],
                    in_=a2p,
                    func=AF.Exp,
                    accum_out=zp[:, sc:sc + 1],
                )
                # chunked weighted-sum accumulation: keeps the drain short
                # and spreads DVE load across the whole MLP phase.
                stt_t = stt_pool.tile([DH, SCW], bf16)
                nc.vector.scalar_tensor_tensor(
                    out=stt_t,
                    in0=e_t[:, SCW * hc:SCW * (hc + 1)],
                    scalar=1.0,
                    in1=hcur[:, h, SCW * sc:SCW * (sc + 1)],
                    op0=OP.mult,
                    op1=OP.mult,
                    accum_out=v_sb[:, col, sc:sc + 1],
                )

            def mlp_finish(b, h):
                col = b * NH + h
                z1 = small_pool.tile([DH, 1], f32, tag="z1")
                nc.vector.tensor_reduce(
                    out=z1, in_=small_state[(b, h)], axis=AX.X, op=OP.add
                )
                nc.vector.reciprocal(zr_sb[:, col:col + 1], z1)

            # --- ratio-paced scheduler over quarter-granularity units ---
            # P units: (b, dt, sc) projection quarters; M units: (b, h, sc)
            # MLP quarters. Pace M:P at 32:24 per batch so ACT/DVE load
            # stays near its average; M gated on its head's remaps.
            dts_of = {}
            for h in range(NH):
                dts_of[h] = sorted({(DH * h) // 128, (DH * h + DH - 1) // 128})
            # b0 runs in sc-pair-major order so it only needs the first-half
            # xt columns for its first 12 P-units; later batches are fully
            # prefetched, so plain dt-major order is fine there. (Pair-major
            # for the last batch was tried and measured ~2.4us worse: the
            # batched M backlog floods ACT/DVE and stalls the PE.)
            P_units = [(0, dt, 2 * sp + s) for sp in range(2)
                       for dt in range(DT) for s in range(2)]
            P_units += [(b, dt, sc) for b in range(1, BPC) for dt in range(DT)
                        for sc in range(SC)]
            M_units = [(0, h, 2 * sp + s) for sp in range(2)
                       for h in range(NH) for s in range(2)]
            M_units += [(b, h, sc) for b in range(1, BPC) for h in range(NH)
                        for sc in range(SC)]
            flat_state = {}
            small_state = {}
            half_state = {}
            remapped = set()
            xt_tiles = {0: xt_t0}
            p_i = m_i = 0

            def emit_P():
                nonlocal p_i
                b, dt, sc = P_units[p_i]
                # prefetch b+1: two kt-DMAs at each of dt 2,3,4 so the Sync
                # engine's ~0.75us/descriptor cost doesn't burst-delay remaps
                if dt in (2, 3, 4) and sc == 0 and b + 1 < BPC:
                    if dt == 2:
                        xt_tiles[b + 1] = xt_pool.tile(
                            [128, KT, S], bf16, tag="xt0", name="nxt"
                        )
                    nxt = xt_tiles[b + 1]
                    for kt in (2 * (dt - 2), 2 * (dt - 2) + 1):
                        nc.sync.dma_start(out=nxt[:, kt, :], in_=xt[b + 1, kt])
                if sc == 0:
                    flat_state[(b, dt)] = flat_pool.tile([128, S], bf16, tag="flat", name="flat_t")
                proj_quarter(b, xt_tiles[b], dt, sc, flat_state[(b, dt)])
                if sc % 2 == 1:
                    emit_remap_dtile(b, dt, sc // 2, flat_state[(b, dt)])
                    remapped.add((b, dt, sc // 2))
                if sc == SC - 1:
                    flat_state.pop((b, dt))
                p_i += 1

            def emit_M():
                nonlocal m_i
                b, h, sc = M_units[m_i]
                if sc == 0:
                    # zp spans all 4 quarters; up to ~9 live in b0's
                    # pair-major order, hence the explicit bufs.
                    small_state[(b, h)] = small_pool.tile(
                        [DH, SC], f32, tag="zp", name="zp", bufs=12
                    )
                if sc % 2 == 0:
                    # e tile covers one half-S: keeps SBUF flat even when
                    # b0 interleaves all 8 heads pair-major.
                    half_state[(b, h)] = e_pool.tile(
                        [DH, 2 * SCW], bf16, tag="e_t", name="e_t"
                    )
                e_t = half_state[(b, h)]
                zp = small_state[(b, h)]
                mlp_quarter(b, h, sc, e_t, zp, flush=(p_i >= len(P_units)))
                if sc % 2 == 1:
                    del half_state[(b, h)]
                if sc == SC - 1:
                    mlp_finish(b, h)
                    del small_state[(b, h)]
                m_i += 1

            def m_ready():
                if m_i >= len(M_units):
                    return False
                b, h, sc = M_units[m_i]
                return all((b, dt, sc // 2) in remapped for dt in dts_of[h])

            while p_i < len(P_units) or m_i < len(M_units):
                lead = LEAD if p_i < len(P_units) - 16 else 0
                # pace M slightly faster than the 4:3 steady-state ratio so
                # the engine-bound M backlog is small when P runs out; much
                # beyond this floods ACT/DVE mid-run and stalls the PE
                want_m = m_i * 13 <= (p_i - lead) * 18
                if p_i < len(P_units) and not (want_m and m_ready()):
                    emit_P()
                elif m_ready():
                    emit_M()
                elif p_i < len(P_units):
                    emit_P()
                else:
                    # only unready M left: emit in order anyway (deps safe)
                    emit_M()

            # v = (sum of per-quarter accums) / Z, then transpose (96, 32)
            # -> (32, 96) and store.
            vq_sb = singles.tile([DH, BPC * NH], f32)
            nc.vector.tensor_reduce(
                out=vq_sb, in_=v_sb, axis=AX.X, op=OP.add
            )
            vn_sb = singles.tile([DH, BPC * NH], f32)
            nc.vector.tensor_mul(vn_sb, vq_sb, zr_sb)
            vout_p = a2p_pool.tile([BPC * NH, DH], f32, tag="a2p")
            nc.tensor.matmul(vout_p, vn_sb, id_sb, start=True, stop=True)
            out_sb = singles.tile([BPC * NH, DH], f32)
            nc.scalar.copy(out=out_sb, in_=vout_p)
            nc.sync.dma_start(out=out[:], in_=out_sb)
    nc.compile()
    return nc


def get_nc():
    if "nc" not in _NC_CACHE:
        _NC_CACHE["nc"] = _build_nc()
    return _NC_CACHE["nc"]


def make_in_maps(token_embeddings, P_w, P_b, W1_w, W1_b, W2_w, W2_b):
    import ml_dtypes

    bf16 = ml_dtypes.bfloat16
    X = np.asarray(token_embeddings, dtype=np.float32)
    # X^T per batch: (B, T, S) -> tiles [b, kt, p, s]
    XT = np.ascontiguousarray(X.transpose(0, 2, 1)).astype(bf16)
    XT = XT.reshape(B, KT, 128, S)

    P_cat = np.transpose(np.asarray(P_w, np.float32), (1, 0, 2)).reshape(T, NH * DH)
    p_l = np.ascontiguousarray(
        P_cat.reshape(KT, 128, NH * DH).transpose(1, 0, 2)
    ).astype(bf16)

    w1 = np.ascontiguousarray(
        np.asarray(W1_w, np.float32).transpose(1, 0, 2)
    ).astype(bf16)
    b1l = np.ascontiguousarray(
        np.asarray(W1_b, np.float32).reshape(NH, KC, 128).transpose(2, 0, 1)
    ).astype(np.float32)

    w2 = np.ascontiguousarray(
        np.asarray(W2_w, np.float32).reshape(NH, KC, 128, DH).transpose(2, 0, 1, 3)
    ).astype(bf16)

    pb = np.ascontiguousarray(
        np.asarray(P_b, np.float32).reshape(NH * DH).reshape(KT, 128).T
    ).astype(np.float32)
    ident = np.eye(DH, dtype=np.float32)

    in_maps = []
    for c in range(NCORES):
        in_maps.append({
            "xt": np.ascontiguousarray(XT[c * BPC:(c + 1) * BPC]),
            "p_l": p_l,
            "w1": w1,
            "b1l": b1l,
            "w2": w2,
            "pb": pb,
            "ident": ident,
        })
    return in_maps


def _reference_host(token_embeddings, attention_mask, P_w, P_b, W1_w, W1_b, W2_w, W2_b):
    """Exact numpy fallback (only used if the mask is not all-ones)."""
    X = np.asarray(token_embeddings, np.float64)
    Hi = np.einsum("bst,htd->bhsd", X, np.asarray(P_w, np.float64))
    Hi += np.asarray(P_b, np.float64)[None, :, None, :]
    A = np.einsum("bhsd,hde->bhse", Hi, np.asarray(W1_w, np.float64))
    A += np.asarray(W1_b, np.float64)[None, :, None, :]
    A = np.maximum(A, 0.0)
    A = np.einsum("bhse,hed->bhsd", A, np.asarray(W2_w, np.float64))
    A += np.asarray(W2_b, np.float64)[None, :, None, :]
    with np.errstate(divide="ignore"):
        logm = np.log(np.asarray(attention_mask, np.float64))[:, None, :, None]
    A = A + logm
    A = A - A.max(axis=2, keepdims=True)
    E = np.exp(A)
    A = E / E.sum(axis=2, keepdims=True)
    v = (Hi * A).sum(axis=2)
    return v.reshape(v.shape[0], NH * DH).astype(np.float32)


def kernel(**inputs):
    mask = np.asarray(inputs["attention_mask"], np.float32)
    if not np.all(mask == 1.0):
        return _reference_host(**inputs)

    from concourse.bass_utils import run_bass_kernel_spmd

    nc = get_nc()
    in_maps = make_in_maps(
        inputs["token_embeddings"], inputs["P_w"], inputs["P_b"],
        inputs["W1_w"], inputs["W1_b"], inputs["W2_w"], inputs["W2_b"],
    )
    res = run_bass_kernel_spmd(nc, in_maps, core_ids=list(range(NCORES)))
    outs = [
        np.asarray(r["out"], np.float32).reshape(BPC, NH * DH)
        for r in res.results
    ]
    return np.concatenate(outs, axis=0)

